# revision 7
# baseline (speedup 1.0000x reference)
"""Causal self-attention (B=2, T=2048, D=1024, H=16) on 8 TRN2 NeuronCores.

Sharding: data-parallel over batch (2) x tensor-parallel over head groups (4),
so each core handles one batch element and 4 heads (256 of the 1024 attention
channels). The out-projection is row-sharded; the host sums the 4 partial
outputs per batch element.

Per-core kernel (all fp32):
  - host supplies x^T [D, T] so the contraction dim is on partitions
  - Q^T/K^T computed in [o, t] layout (lhsT = W slice, rhs = x^T);
    V in [t, o] layout (lhsT = x^T tile, rhs = Wv slice)
  - attention in the transposed orientation: S^T tiles [128 k, 512 q] =
    K^T_tile.T @ Q^T_strip (contraction = head dim 64); exp on ScalarE
    (no max subtraction -- scores are O(1) for this input distribution);
    causal wedge zeroed with gpsimd.affine_select
  - P^T @ ... : out^T[o, q] accumulates over key tiles with lhsT = [V | 1]
    so row 64 of the PSUM accumulator is the softmax denominator l[q]
  - normalization: r = 1/l on VectorE, broadcast across partitions via a
    DRAM round-trip DMA, one tensor_mul per head
  - out-proj: lhsT = attn^T tiles, rhs = W_out rows for this head group
Bias handling: b_k dropped (softmax shift-invariant per query), b_q applied
via per-partition tensor_scalar_add, b_v and b_out folded into a host-side
constant (softmax rows sum to 1).
"""

import numpy as np

B, T_FULL, D, H = 2, 2048, 1024, 16
DH = 64
HC = 4            # heads per core
OC = HC * DH      # 256 attention channels per core
NCORES = 8


def build_nc(T=T_FULL):
    import concourse.bass as bass
    import concourse.mybir as mybir
    from concourse import bacc
    from concourse.tile import TileContext

    f32 = mybir.dt.float32
    AF = mybir.ActivationFunctionType
    ALU = mybir.AluOpType

    KD = D // 128           # 8 contraction tiles for the projections
    TT = T // 128           # token tiles
    TCH = T // 512          # token chunks of 512
    NS = T // 512           # query strips of 512
    KO = OC // 128          # 2 o-tiles for Q/K (and out-proj contraction)

    nc = bacc.Bacc("TRN2", target_bir_lowering=False)
    xT_d = nc.dram_tensor("xT", [D, T], f32, kind="ExternalInput")
    wq_d = nc.dram_tensor("wq", [D, OC], f32, kind="ExternalInput")
    wk_d = nc.dram_tensor("wk", [D, OC], f32, kind="ExternalInput")
    wv_d = nc.dram_tensor("wv", [D, OC], f32, kind="ExternalInput")
    bq_d = nc.dram_tensor("bq", [OC], f32, kind="ExternalInput")
    wo_d = nc.dram_tensor("wo", [OC, D], f32, kind="ExternalInput")
    out_d = nc.dram_tensor("out", [T, D], f32, kind="ExternalOutput")
    r_dram = nc.dram_tensor("r_scratch", [HC, T], f32)

    with TileContext(nc) as tc:
        with (
            tc.tile_pool(name="persist", bufs=1) as P1,
            tc.tile_pool(name="work", bufs=3) as WK,
            tc.tile_pool(name="rbp", bufs=2) as RBP,
        ):
            QT = P1.tile([128, KO, T], f32)
            KT = P1.tile([128, KO, T], f32)
            V = P1.tile([128, TT, HC, DH + 1], f32)
            attnT = P1.tile([128, KO, T], f32)
            wo = P1.tile([128, KO, D], f32)
            nc.sync.dma_start(wo[:], wo_d[:].rearrange("(k p) n -> p k n", p=128))

            # ---------------- QKV projections ----------------
            with (
                tc.tile_pool(name="qkv_in", bufs=1) as QI,
                tc.tile_pool(name="psA", bufs=4, space="PSUM") as PSA,
                tc.tile_pool(name="psB", bufs=2, space="PSUM") as PSB,
            ):
                xT = QI.tile([128, KD, T], f32)
                xT_r = xT_d[:].rearrange("(k p) t -> p k t", p=128)
                for k in range(KD):
                    nc.sync.dma_start(xT[:, k, :], xT_r[:, k, :])
                wq = QI.tile([128, KD, OC], f32)
                wk = QI.tile([128, KD, OC], f32)
                wv = QI.tile([128, KD, OC], f32)
                bq = QI.tile([128, KO], f32)
                nc.sync.dma_start(wq[:], wq_d[:].rearrange("(k p) o -> p k o", p=128))
                nc.sync.dma_start(wk[:], wk_d[:].rearrange("(k p) o -> p k o", p=128))
                nc.sync.dma_start(wv[:], wv_d[:].rearrange("(k p) o -> p k o", p=128))
                nc.sync.dma_start(bq[:], bq_d[:].rearrange("(o p) -> p o", p=128))

                for ot in range(KO):
                    for tch in range(TCH):
                        ps = PSA.tile([128, 512], f32, tag="pqk")
                        for k in range(KD):
                            nc.tensor.matmul(
                                ps[:],
                                wq[:, k, ot * 128:(ot + 1) * 128],
                                xT[:, k, tch * 512:(tch + 1) * 512],
                                start=(k == 0), stop=(k == KD - 1),
                            )
                        nc.scalar.activation(
                            QT[:, ot, tch * 512:(tch + 1) * 512], ps[:],
                            AF.Identity, bias=bq[:, ot:ot + 1],
                        )
                    for tch in range(TCH):
                        ps = PSA.tile([128, 512], f32, tag="pqk")
                        for k in range(KD):
                            nc.tensor.matmul(
                                ps[:],
                                wk[:, k, ot * 128:(ot + 1) * 128],
                                xT[:, k, tch * 512:(tch + 1) * 512],
                                start=(k == 0), stop=(k == KD - 1),
                            )
                        nc.vector.tensor_copy(KT[:, ot, tch * 512:(tch + 1) * 512], ps[:])

                for tt in range(TT):
                    ps = PSB.tile([128, OC], f32, tag="pvv")
                    for k in range(KD):
                        nc.tensor.matmul(
                            ps[:],
                            xT[:, k, tt * 128:(tt + 1) * 128],
                            wv[:, k, :],
                            start=(k == 0), stop=(k == KD - 1),
                        )
                    nc.vector.tensor_copy(
                        V[:, tt, :, 0:DH], ps[:].rearrange("p (h o) -> p h o", h=HC)
                    )
                nc.gpsimd.memset(V[:, :, :, DH:DH + 1], 1.0)

            # ---------------- attention ----------------
            with (
                tc.tile_pool(name="psC", bufs=4, space="PSUM") as PSC,
                tc.tile_pool(name="psD", bufs=2, space="PSUM") as PSD,
            ):
                for h in range(HC):
                    hp, po = h // 2, (h % 2) * 64
                    lf = WK.tile([TT, 128], f32, tag="lf")
                    for s in range(NS):
                        nk = 4 * (s + 1)
                        pso = PSD.tile([DH + 1, 512], f32, tag="po")
                        for ki in range(nk):
                            pss = PSC.tile([128, 512], f32, tag="ps")
                            nc.tensor.matmul(
                                pss[:],
                                KT[po:po + 64, hp, ki * 128:(ki + 1) * 128],
                                QT[po:po + 64, hp, s * 512:(s + 1) * 512],
                                start=True, stop=True,
                            )
                            pt = WK.tile([128, 512], f32, tag="pt")
                            nc.scalar.activation(pt[:], pss[:], AF.Exp, scale=0.125)
                            if ki >= nk - 4:
                                kil = ki - (nk - 4)
                                nc.gpsimd.affine_select(
                                    pt[:], pt[:], pattern=[[1, 512]],
                                    compare_op=ALU.is_ge, fill=0.0,
                                    base=-128 * kil, channel_multiplier=-1,
                                )
                            nc.tensor.matmul(
                                pso[:], V[:, ki, h, :], pt[:],
                                start=(ki == 0), stop=(ki == nk - 1),
                            )
                        nc.vector.tensor_copy(
                            attnT[po:po + 64, hp, s * 512:(s + 1) * 512],
                            pso[0:DH, :],
                        )
                        ls = WK.tile([1, 512], f32, tag="ls")
                        nc.vector.tensor_copy(ls[:], pso[DH:DH + 1, :])
                        nc.sync.dma_start(
                            lf[4 * s:4 * s + 4, :],
                            ls[:].rearrange("o (j c) -> o j c", j=4),
                        )
                    rf = WK.tile([TT, 128], f32, tag="rf")
                    nc.vector.reciprocal(rf[:], lf[:])
                    nc.sync.dma_start(r_dram[h:h + 1, :], rf[:])
                    # broadcast r into the partition range matching this head
                    # (walrus requires equal base partitions for SB+SB ops)
                    rb = RBP.tile([128, T], f32, tag="rb")
                    nc.sync.dma_start(
                        rb[po:po + 64, :], bass.AP(r_dram, h * T, [[0, 64], [1, T]])
                    )
                    nc.vector.tensor_mul(
                        attnT[po:po + 64, hp, :], attnT[po:po + 64, hp, :],
                        rb[po:po + 64, :],
                    )

            # ---------------- out projection ----------------
            with tc.tile_pool(name="psE", bufs=2, space="PSUM") as PSE:
                for tt in range(TT):
                    ps = PSE.tile([128, D], f32, tag="pp")
                    for nch in range(2):
                        for k2 in range(KO):
                            nc.tensor.matmul(
                                ps[:, nch * 512:(nch + 1) * 512],
                                attnT[:, k2, tt * 128:(tt + 1) * 128],
                                wo[:, k2, nch * 512:(nch + 1) * 512],
                                start=(k2 == 0), stop=(k2 == KO - 1),
                            )
                    st = WK.tile([128, D], f32, tag="st")
                    nc.vector.tensor_copy(st[:, 0:512], ps[:, 0:512])
                    nc.scalar.copy(st[:, 512:D], ps[:, 512:D])
                    nc.sync.dma_start(out_d[tt * 128:(tt + 1) * 128, :], st[:])

    if hasattr(nc, "compile"):
        nc.compile()
    return nc


def shard_inputs(x, w_qkv, b_qkv, w_out):
    """Build the 8 per-core input dicts (core = b * 4 + g)."""
    in_maps = []
    for core in range(NCORES):
        b, g = core // 4, core % 4
        o0 = g * OC
        in_maps.append({
            "xT": np.ascontiguousarray(np.asarray(x[b]).T, dtype=np.float32),
            "wq": np.ascontiguousarray(w_qkv[:, o0:o0 + OC], dtype=np.float32),
            "wk": np.ascontiguousarray(w_qkv[:, D + o0:D + o0 + OC], dtype=np.float32),
            "wv": np.ascontiguousarray(w_qkv[:, 2 * D + o0:2 * D + o0 + OC], dtype=np.float32),
            "bq": np.ascontiguousarray(b_qkv[o0:o0 + OC], dtype=np.float32),
            "wo": np.ascontiguousarray(w_out[o0:o0 + OC, :], dtype=np.float32),
        })
    return in_maps


_NC_CACHE = {}


def kernel(x, w_qkv, b_qkv, w_out, b_out):
    from concourse.bass_utils import run_bass_kernel_spmd

    x = np.asarray(x, dtype=np.float32)
    w_qkv = np.asarray(w_qkv, dtype=np.float32)
    b_qkv = np.asarray(b_qkv, dtype=np.float32)
    w_out = np.asarray(w_out, dtype=np.float32)
    b_out = np.asarray(b_out, dtype=np.float32)

    if "nc" not in _NC_CACHE:
        _NC_CACHE["nc"] = build_nc(T_FULL)
    nc = _NC_CACHE["nc"]

    in_maps = shard_inputs(x, w_qkv, b_qkv, w_out)
    res = run_bass_kernel_spmd(nc, in_maps, list(range(NCORES)))

    # b_v and b_out folded here: softmax rows sum to 1, so the v-bias
    # contributes b_v @ w_out to every token.
    b_eff = (b_out + b_qkv[2 * D:] @ w_out).astype(np.float32)
    out = np.empty((B, T_FULL, D), dtype=np.float32)
    for b in range(B):
        acc = res.results[b * 4]["out"].astype(np.float32)
        for g in range(1, 4):
            acc = acc + res.results[b * 4 + g]["out"]
        out[b] = acc + b_eff
    return out
